# revision 31
# baseline (speedup 1.0000x reference)
"""Trainium2 Bass kernel for nn_Erosion2d (sequential morphological erosion).

Bit-exact reformulation of the reference:
  x stays binary {0,1} throughout; every structuring element is rank-1
  (separable), so each erosion = separable AND (H-pass then V-pass) and each
  dilation = separable OR. The whole pipeline runs on 32x bit-packed words
  (int32; bit b of word (r,cw) = pixel (r, 32*cw+b)) using DVE bitwise ops.
  Final stage: result = (3x3 box count of output == 1), computed with
  carry-save bit tricks on packed words.

Layout per image: [128 partitions, j, 32 words]; partition p holds rows
8p..8p+7 at j=1..8 with halo rows j=0 (row 8p-1) and j=9 (row 8p+8).
V-taps are word-aligned free-dim offsets; halos are refreshed with small
SBUF->SBUF partition-shifted DMAs. H-taps are in-word funnel shifts.

Sharding: pure data parallel - 16 images, 2 per NeuronCore on 8 cores.
"""
import numpy as np

import concourse.bass as bass
import concourse.bacc as bacc
import concourse.mybir as mybir
import concourse.tile as tile
from concourse.bass_utils import run_bass_kernel_spmd

Alu = mybir.AluOpType
F32 = mybir.dt.float32
I32 = mybir.dt.int32

N_CORES = 8
IMGS_PER_CORE = 2
H = 1024
W = 1024
P = 128
R = H // P            # 8 rows per partition
WW = W // 32          # 32 words per row
M31 = 0x7FFFFFFF

# separable patterns (tap at -1, 0, +1) for the 8 structuring elements
KV = [(1, 1, 1), (1, 1, 1), (1, 1, 0), (0, 1, 1), (1, 1, 1), (0, 1, 0), (1, 1, 0), (0, 1, 0)]
KH = [(1, 1, 1), (0, 1, 1), (1, 1, 1), (0, 1, 1), (0, 1, 0), (1, 1, 1), (0, 1, 0), (1, 1, 0)]


def _stt_i(eng, out, in0, scalar, in1, op0, op1):
    """scalar_tensor_tensor with an int32-typed immediate."""
    return eng.add_instruction(
        mybir.InstTensorScalarPtr(
            name=eng.bass.get_next_instruction_name(),
            is_scalar_tensor_tensor=True,
            op0=op0, op1=op1,
            ins=[eng.lower_ap(in0),
                 eng.lower_ap_or_imm(scalar, imm_dtype=I32),
                 eng.lower_ap(in1)],
            outs=[eng.lower_ap(out)],
        ))


class _Img:
    """Per-image persistent tiles."""

    def __init__(self, pool, big_pool, idx):
        self.idx = idx
        dt = I32
        self.x = pool.tile([P, R + 2, WW], dt, tag=f"x{idx}")
        self.ero = pool.tile([P, R + 2, WW], dt, tag=f"ero{idx}")
        self.hm = pool.tile([P, R + 2, WW], dt, tag=f"hm{idx}")
        self.out = pool.tile([P, R + 2, WW], dt, tag=f"out{idx}")
        self.c = pool.tile([P, R + 2, WW], dt, tag=f"c{idx}")     # L-carries ([:,:,0]=0)
        self.t = pool.tile([P, R + 2, WW], dt, tag=f"t{idx}")     # scratch
        self.u = pool.tile([P, R + 2, WW], dt, tag=f"u{idx}")     # R-carry ([:,:,31]=0)
        self.w = pool.tile([P, R + 2, WW], dt, tag=f"w{idx}")     # scratch
        self.vis = pool.tile([P, R, WW], dt, tag=f"vis{idx}")
        self.res = pool.tile([P, R, WW], dt, tag=f"res{idx}")
        # big staging buffers are shared across images via tag rotation:
        # stage gets 2 slots (bufs=2 pool), the rest share single slots with
        # WAR deps inserted by Tile.
        self.stage = big_pool.tile([P, R, W], F32, tag="stage")


def _hstage(nc, im, dst, src, pat, kind, jlo=0, jhi=R + 2):
    """H-pass over j-slots [jlo, jhi): dst = src <&/|> shifted taps.

    'ero': dst = src & [L(src) if pat[0]] & [R(src) if pat[2]]
    'dil': dst = src | [R(src) if pat[0]] | [L(src) if pat[2]]
    L(x)[c] = x[c-1] = (w << 1) | (prev word bit31);
    R(x)[c] = x[c+1] = ((w >> 1) & M31) | (next word bit0 << 31).
    src is only read before the single dst write, so dst may alias src.
    Returns the result tile (src itself when the pattern is 010).
    """
    a, _, b = pat
    want_L, want_R = (a, b) if kind == 'ero' else (b, a)
    fold = Alu.bitwise_and if kind == 'ero' else Alu.bitwise_or
    if not (want_L or want_R):
        return src
    v = nc.vector
    S = (slice(None), slice(jlo, jhi), slice(None))
    chains = []
    if want_L:
        v.tensor_scalar(im.c[:, jlo:jhi, 1:WW], src[:, jlo:jhi, 0:WW - 1], 0, None, Alu.is_lt)
        _stt_i(v, im.t[S], src[S], 1, im.c[S], Alu.logical_shift_left, Alu.bitwise_or)
        chains.append(im.t)
    if want_R:
        hold = im.w if want_L else im.t
        v.tensor_scalar(hold[S], src[S], 1, M31, Alu.logical_shift_right, Alu.bitwise_and)
        # (next word << 31) isolates its bit0 at position 31 -- no mask needed;
        # last word untouched: its bit31 stays 0 (right image edge).
        _stt_i(v, hold[:, jlo:jhi, 0:WW - 1], src[:, jlo:jhi, 1:WW], 31,
               hold[:, jlo:jhi, 0:WW - 1], Alu.logical_shift_left, Alu.bitwise_or)
        chains.append(hold)
    if len(chains) == 2:
        v.tensor_tensor(im.t[S], im.t[S], im.w[S], fold)
    v.tensor_tensor(dst[S], chains[0][S], src[S], fold)
    return dst


def _vstage(nc, dst, src, pat, kind):
    """V-pass into payload rows (j=1..8 of a halo'd dst, or all of [P,R,WW]).

    'ero': AND over taps (pat[0] -> row-1, pat[2] -> row+1)
    'dil': OR over taps (pat[0] -> row+1, pat[2] -> row-1)
    src must have valid halo rows. Returns src itself when pattern is 010.
    """
    a, _, b = pat
    up, dn = (a, b) if kind == 'ero' else (b, a)
    op = Alu.bitwise_and if kind == 'ero' else Alu.bitwise_or
    if not (up or dn):
        return src
    v = nc.vector
    U = src[:, 0:R, :]
    C = src[:, 1:R + 1, :]
    D = src[:, 2:R + 2, :]
    payload = dst[:, 1:R + 1, :] if dst.shape[1] == R + 2 else dst[:, :, :]
    if up and dn:
        v.tensor_tensor(payload, C, U, op)
        v.tensor_tensor(payload, payload, D, op)
    elif up:
        v.tensor_tensor(payload, C, U, op)
    else:
        v.tensor_tensor(payload, C, D, op)
    return dst


def _exchange_halo(nc, tl, top=True, bot=True):
    """Refresh halo rows of a [P, R+2, WW] tile via partition-shifted DMA.

    The two directions go on the two HWDGE queues so their ~1.7us init
    latencies overlap."""
    if top:
        # top halo j=0 of partition p = row 8p-1 = partition p-1's last payload j=8
        nc.sync.dma_start(tl[1:P, 0:1, :], tl[0:P - 1, R:R + 1, :])
    if bot:
        # bottom halo j=9 of partition p = row 8p+8 = partition p+1's first payload j=1
        nc.scalar.dma_start(tl[0:P - 1, R + 1:R + 2, :], tl[1:P, 1:2, :])



def _final_stage(nc, im):
    """result = (3x3 box count of out == 1) on packed words, into im.res."""
    v = nc.vector
    U = im.out[:, 0:R, :]
    C = im.out[:, 1:R + 1, :]
    D = im.out[:, 2:R + 2, :]
    v1 = im.x[:, 1:R + 1, :]
    v2 = im.ero[:, 1:R + 1, :]
    s1 = im.hm[:, 1:R + 1, :]
    q1 = im.vis[:, :, :]
    v.tensor_tensor(s1, U, C, Alu.bitwise_xor)
    v.tensor_tensor(q1, s1, D, Alu.bitwise_and)
    v.tensor_tensor(v1, s1, D, Alu.bitwise_xor)
    v.tensor_tensor(s1, U, C, Alu.bitwise_and)
    v.tensor_tensor(v2, s1, q1, Alu.bitwise_or)

    jlo, jhi = 1, R + 1
    S = (slice(None), slice(jlo, jhi), slice(None))
    v1L = im.hm[S]
    v.tensor_scalar(im.c[:, jlo:jhi, 1:WW], im.x[:, jlo:jhi, 0:WW - 1], 0, None, Alu.is_lt)
    _stt_i(v, v1L, v1, 1, im.c[S], Alu.logical_shift_left, Alu.bitwise_or)
    v1R = im.t[S]
    v.tensor_scalar(v1R, v1, 1, M31, Alu.logical_shift_right, Alu.bitwise_and)
    _stt_i(v, im.t[:, jlo:jhi, 0:WW - 1], im.x[:, jlo:jhi, 1:WW], 31,
           im.t[:, jlo:jhi, 0:WW - 1], Alu.logical_shift_left, Alu.bitwise_or)

    v2L = im.w[S]
    v.tensor_scalar(im.c[:, jlo:jhi, 1:WW], im.ero[:, jlo:jhi, 0:WW - 1], 0, None, Alu.is_lt)
    _stt_i(v, v2L, v2, 1, im.c[S], Alu.logical_shift_left, Alu.bitwise_or)
    v2R = im.u[S]
    v.tensor_scalar(v2R, v2, 1, M31, Alu.logical_shift_right, Alu.bitwise_and)
    _stt_i(v, im.u[:, jlo:jhi, 0:WW - 1], im.ero[:, jlo:jhi, 1:WW], 31,
           im.u[:, jlo:jhi, 0:WW - 1], Alu.logical_shift_left, Alu.bitwise_or)

    # any ">= 2" evidence: a v2 flag, or a pair among the v1 flags. With the
    # XOR3 parity, excluding just (v1L & v1) suffices: pairs kill the parity,
    # the triple is killed by the (v1L & v1) term.
    A = im.res[:, :, :]
    v.tensor_tensor(A, v2, v2L, Alu.bitwise_or)
    v.tensor_tensor(A, A, v2R, Alu.bitwise_or)
    P1 = im.vis[:, :, :]
    v.tensor_tensor(P1, v1L, v1, Alu.bitwise_and)
    v.tensor_tensor(A, A, P1, Alu.bitwise_or)
    X = im.hm[:, 1:R + 1, :]
    v.tensor_tensor(X, v1L, v1, Alu.bitwise_xor)
    v.tensor_tensor(X, X, v1R, Alu.bitwise_xor)
    _stt_i(v, im.res[:, :, :], A, -1, X, Alu.bitwise_xor, Alu.bitwise_and)


def build_program():
    nc = bacc.Bacc("TRN2", target_bir_lowering=False, debug=False, num_devices=N_CORES)
    x_in = nc.dram_tensor("x", [IMGS_PER_CORE, H, W], F32, kind="ExternalInput")
    y_out = nc.dram_tensor("y", [IMGS_PER_CORE, H, W], F32, kind="ExternalOutput")
    with tile.TileContext(nc) as tc:
        _emit(tc, x_in, y_out)
    nc.compile()
    return nc


def _emit(tc, x_in, y_out):
    nc = tc.nc
    if True:
        with tc.tile_pool(name="small", bufs=1) as pool, \
             tc.tile_pool(name="big", bufs=2) as big_pool, \
             tc.tile_pool(name="scratch", bufs=1) as scratch_pool:
            ims = [_Img(pool, big_pool, i) for i in range(IMGS_PER_CORE)]
            v = nc.vector

            # ---- one-time edge zeroing (image-boundary halo invariants) ----
            for im in ims:
                v.memset(im.x[0:32, 0, :], 0)
                v.memset(im.x[96:P, R + 1, :], 0)
                v.memset(im.ero[0:32, 0, :], 0)
                v.memset(im.ero[96:P, R + 1, :], 0)
                v.memset(im.out[:], 0)              # output accumulator + halos
                v.memset(im.c[:, :, 0:1], 0)        # L-carry left edge

            # ---- load (column-chunked) + pack, pipelined per chunk ----
            CHUNKS = [(c, c + 256) for c in range(0, W, 256)]
            NCH = len(CHUNKS)
            pk1s, pk2s, pk3s, pk4s = {}, {}, {}, {}
            for im in ims:
                pk1s[im] = scratch_pool.tile([P, R, W // 2], F32, tag=f"pka{im.idx}", name=f"pka{im.idx}")
                pk2s[im] = scratch_pool.tile([P, R, W // 4], F32, tag=f"pkb{im.idx}", name=f"pkb{im.idx}")
                pk3s[im] = scratch_pool.tile([P, R, W // 8], F32, tag=f"pkc{im.idx}", name=f"pkc{im.idx}")
                pk4s[im] = scratch_pool.tile([P, R, W // 16], I32, tag=f"pkd{im.idx}", name=f"pkd{im.idx}")
            for im, xi in zip(ims, range(IMGS_PER_CORE)):
                xr = x_in[xi].rearrange("(p j) c -> p j c", j=R)
                for ci, (c0, c1) in enumerate(CHUNKS):
                    eng = nc.sync if ci % 2 == 0 else nc.scalar
                    eng.dma_start(im.stage[:, :, c0:c1], xr[:, :, c0:c1])
                    st, pk1, pk2, pk3, pk4 = im.stage, pk1s[im], pk2s[im], pk3s[im], pk4s[im]
                    v.scalar_tensor_tensor(pk1[:, :, c0 // 2:c1 // 2], st[:, :, c0 + 1:c1:2], 2.0,
                                           st[:, :, c0:c1:2], Alu.mult, Alu.add)
                    v.scalar_tensor_tensor(pk2[:, :, c0 // 4:c1 // 4], pk1[:, :, c0 // 2 + 1:c1 // 2:2], 4.0,
                                           pk1[:, :, c0 // 2:c1 // 2:2], Alu.mult, Alu.add)
                    v.scalar_tensor_tensor(pk3[:, :, c0 // 8:c1 // 8], pk2[:, :, c0 // 4 + 1:c1 // 4:2], 16.0,
                                           pk2[:, :, c0 // 4:c1 // 4:2], Alu.mult, Alu.add)
                    # L4 writes int32 directly (arithmetic ops may cast)
                    v.scalar_tensor_tensor(pk4[:, :, c0 // 16:c1 // 16], pk3[:, :, c0 // 8 + 1:c1 // 8:2], 256.0,
                                           pk3[:, :, c0 // 8:c1 // 8:2], Alu.mult, Alu.add)
                    _stt_i(v, im.x[:, 1:R + 1, c0 // 32:c1 // 32], pk4[:, :, c0 // 16 + 1:c1 // 16:2], 16,
                           pk4[:, :, c0 // 16:c1 // 16:2], Alu.logical_shift_left, Alu.bitwise_or)
            for im in ims:
                _exchange_halo(nc, im.x)

            # ---- 8 sequential erosion/dilation iterations ----
            ero_ts = {}
            for k in range(8):
                vpat, hpat = KV[k], KH[k]
                # halo rows needed from the ero result by the dilation V-pass
                d_top, d_bot = (vpat[2], vpat[0]) if (vpat[0] or vpat[2]) else (0, 0)
                # halo rows of x needed by the NEXT iteration's erosion V-pass
                if k < 7:
                    nv = KV[k + 1]
                    x_top, x_bot = nv[0], nv[2]
                else:
                    x_top = x_bot = 0
                for im in ims:   # erosion phase (interleaved across images)
                    jlo, jhi = 1 - (vpat[0] or 0), R + 1 + (vpat[2] or 0)
                    hsrc = _hstage(nc, im, im.hm, im.x, hpat, 'ero', jlo, jhi)
                    esrc = _vstage(nc, im.ero, hsrc, vpat, 'ero')
                    ero_t = esrc if esrc is hsrc else im.ero
                    if ero_t is im.ero and (d_top or d_bot):
                        _exchange_halo(nc, ero_t, top=bool(d_top), bot=bool(d_bot))
                    ero_ts[im] = ero_t
                for im in ims:   # dilation + update phase
                    ero_t = ero_ts[im]
                    v.tensor_tensor(im.out[:, 1:R + 1, :], im.out[:, 1:R + 1, :],
                                    ero_t[:, 1:R + 1, :], Alu.bitwise_or)
                    if k == 7:
                        # out is final now; exchange its halos early so the
                        # final stage doesn't wait on the DMA init latency
                        _exchange_halo(nc, im.out)
                    jlo, jhi = 1 - (d_top or 0), R + 1 + (d_bot or 0)
                    dsrc = _hstage(nc, im, im.hm, ero_t, hpat, 'dil', jlo, jhi)
                    vsrc = _vstage(nc, im.vis, dsrc, vpat, 'dil')
                    vis_ap = vsrc[:, 1:R + 1, :] if vsrc.shape[1] == R + 2 else vsrc[:, :, :]
                    _stt_i(v, im.x[:, 1:R + 1, :], vis_ap, -1, im.x[:, 1:R + 1, :],
                           Alu.bitwise_xor, Alu.bitwise_and)
                    if x_top or x_bot:
                        _exchange_halo(nc, im.x, top=bool(x_top), bot=bool(x_bot))

            # ---- final (exactly-one of 3x3 box) + unpack/cast/store, per image ----
            for i, im in enumerate(ims):
                _final_stage(nc, im)
                stageI = scratch_pool.tile([P, R, W], I32, tag="stageI")
                # last image: small final chunk so its cast+store tail is short
                bounds = [0, 32] if i == 0 else [0, 13, 24, 32]
                for half in range(len(bounds) - 1):
                    w0, w1 = bounds[half], bounds[half + 1]
                    c0, c1 = w0 * 32, w1 * 32
                    for b in range(32):
                        if b == 0:
                            v.tensor_scalar(stageI[:, :, c0::32][:, :, 0:w1 - w0], im.res[:, :, w0:w1],
                                            1, None, Alu.bitwise_and)
                        else:
                            v.tensor_scalar(stageI[:, :, c0 + b::32][:, :, 0:w1 - w0], im.res[:, :, w0:w1],
                                            b, 1, Alu.logical_shift_right, Alu.bitwise_and)
                    nc.scalar.copy(im.stage[:, :, c0:c1], stageI[:, :, c0:c1])
                    yr = y_out[i].rearrange("(p j) c -> p j c", j=R)
                    seng = nc.sync if half % 2 == 0 else nc.scalar
                    seng.dma_start(yr[:, :, c0:c1], im.stage[:, :, c0:c1])


_PROGRAM = None


def _get_program():
    global _PROGRAM
    if _PROGRAM is None:
        _PROGRAM = build_program()
    return _PROGRAM


def kernel(x: np.ndarray, kernels: np.ndarray = None, **_):
    """x: [16,1,1024,1024] fp32 binary -> [16,1,1024,1024] fp32."""
    x = np.ascontiguousarray(np.asarray(x), dtype=np.float32)
    N = x.shape[0]
    xs = x.reshape(N, H, W)
    nc = _get_program()
    in_maps = [{"x": np.ascontiguousarray(xs[c * IMGS_PER_CORE:(c + 1) * IMGS_PER_CORE])}
               for c in range(N_CORES)]
    res = run_bass_kernel_spmd(nc, in_maps, core_ids=list(range(N_CORES)))
    out = np.concatenate([r["y"] for r in res.results], axis=0)
    return out.reshape(N, 1, H, W).astype(np.float32)


# revision 32
# speedup vs baseline: 1.0098x; 1.0098x over previous
"""Trainium2 Bass kernel for nn_Erosion2d (sequential morphological erosion).

Bit-exact reformulation of the reference:
  x stays binary {0,1} throughout; every structuring element is rank-1
  (separable), so each erosion = separable AND (H-pass then V-pass) and each
  dilation = separable OR. The whole pipeline runs on 32x bit-packed words
  (int32; bit b of word (r,cw) = pixel (r, 32*cw+b)) using DVE bitwise ops.
  Final stage: result = (3x3 box count of output == 1), computed with
  carry-save bit tricks on packed words.

Layout per image: [128 partitions, j, 32 words]; partition p holds rows
8p..8p+7 at j=1..8 with halo rows j=0 (row 8p-1) and j=9 (row 8p+8).
V-taps are word-aligned free-dim offsets; halos are refreshed with small
SBUF->SBUF partition-shifted DMAs. H-taps are in-word funnel shifts.

Sharding: pure data parallel - 16 images, 2 per NeuronCore on 8 cores.
"""
import numpy as np

import concourse.bass as bass
import concourse.bacc as bacc
import concourse.mybir as mybir
import concourse.tile as tile
from concourse.bass_utils import run_bass_kernel_spmd

Alu = mybir.AluOpType
F32 = mybir.dt.float32
I32 = mybir.dt.int32

N_CORES = 8
IMGS_PER_CORE = 2
H = 1024
W = 1024
P = 128
R = H // P            # 8 rows per partition
WW = W // 32          # 32 words per row
M31 = 0x7FFFFFFF

# separable patterns (tap at -1, 0, +1) for the 8 structuring elements
KV = [(1, 1, 1), (1, 1, 1), (1, 1, 0), (0, 1, 1), (1, 1, 1), (0, 1, 0), (1, 1, 0), (0, 1, 0)]
KH = [(1, 1, 1), (0, 1, 1), (1, 1, 1), (0, 1, 1), (0, 1, 0), (1, 1, 1), (0, 1, 0), (1, 1, 0)]


def _stt_i(eng, out, in0, scalar, in1, op0, op1):
    """scalar_tensor_tensor with an int32-typed immediate."""
    return eng.add_instruction(
        mybir.InstTensorScalarPtr(
            name=eng.bass.get_next_instruction_name(),
            is_scalar_tensor_tensor=True,
            op0=op0, op1=op1,
            ins=[eng.lower_ap(in0),
                 eng.lower_ap_or_imm(scalar, imm_dtype=I32),
                 eng.lower_ap(in1)],
            outs=[eng.lower_ap(out)],
        ))


class _Img:
    """Per-image persistent tiles."""

    def __init__(self, pool, big_pool, idx):
        self.idx = idx
        dt = I32
        self.x = pool.tile([P, R + 2, WW], dt, tag=f"x{idx}")
        self.ero = pool.tile([P, R + 2, WW], dt, tag=f"ero{idx}")
        self.hm = pool.tile([P, R + 2, WW], dt, tag=f"hm{idx}")
        self.out = pool.tile([P, R + 2, WW], dt, tag=f"out{idx}")
        self.c = pool.tile([P, R + 2, WW], dt, tag=f"c{idx}")     # L-carries ([:,:,0]=0)
        self.t = pool.tile([P, R + 2, WW], dt, tag=f"t{idx}")     # scratch
        self.u = pool.tile([P, R + 2, WW], dt, tag=f"u{idx}")     # R-carry ([:,:,31]=0)
        self.w = pool.tile([P, R + 2, WW], dt, tag=f"w{idx}")     # scratch
        self.vis = pool.tile([P, R, WW], dt, tag=f"vis{idx}")
        self.res = pool.tile([P, R, WW], dt, tag=f"res{idx}")
        # big staging buffers are shared across images via tag rotation:
        # stage gets 2 slots (bufs=2 pool), the rest share single slots with
        # WAR deps inserted by Tile.
        self.stage = big_pool.tile([P, R, W], F32, tag="stage")


def _hstage(nc, im, dst, src, pat, kind, jlo=0, jhi=R + 2):
    """H-pass over j-slots [jlo, jhi): dst = src <&/|> shifted taps.

    'ero': dst = src & [L(src) if pat[0]] & [R(src) if pat[2]]
    'dil': dst = src | [R(src) if pat[0]] | [L(src) if pat[2]]
    L(x)[c] = x[c-1] = (w << 1) | (prev word bit31);
    R(x)[c] = x[c+1] = ((w >> 1) & M31) | (next word bit0 << 31).
    src is only read before the single dst write, so dst may alias src.
    Returns the result tile (src itself when the pattern is 010).
    """
    a, _, b = pat
    want_L, want_R = (a, b) if kind == 'ero' else (b, a)
    fold = Alu.bitwise_and if kind == 'ero' else Alu.bitwise_or
    if not (want_L or want_R):
        return src
    v = nc.vector
    S = (slice(None), slice(jlo, jhi), slice(None))
    chains = []
    if want_L:
        v.tensor_scalar(im.c[:, jlo:jhi, 1:WW], src[:, jlo:jhi, 0:WW - 1], 0, None, Alu.is_lt)
        _stt_i(v, im.t[S], src[S], 1, im.c[S], Alu.logical_shift_left, Alu.bitwise_or)
        chains.append(im.t)
    if want_R:
        hold = im.w if want_L else im.t
        v.tensor_scalar(hold[S], src[S], 1, M31, Alu.logical_shift_right, Alu.bitwise_and)
        # (next word << 31) isolates its bit0 at position 31 -- no mask needed;
        # last word untouched: its bit31 stays 0 (right image edge).
        _stt_i(v, hold[:, jlo:jhi, 0:WW - 1], src[:, jlo:jhi, 1:WW], 31,
               hold[:, jlo:jhi, 0:WW - 1], Alu.logical_shift_left, Alu.bitwise_or)
        chains.append(hold)
    if len(chains) == 2:
        v.tensor_tensor(im.t[S], im.t[S], im.w[S], fold)
    v.tensor_tensor(dst[S], chains[0][S], src[S], fold)
    return dst


def _vstage(nc, dst, src, pat, kind):
    """V-pass into payload rows (j=1..8 of a halo'd dst, or all of [P,R,WW]).

    'ero': AND over taps (pat[0] -> row-1, pat[2] -> row+1)
    'dil': OR over taps (pat[0] -> row+1, pat[2] -> row-1)
    src must have valid halo rows. Returns src itself when pattern is 010.
    """
    a, _, b = pat
    up, dn = (a, b) if kind == 'ero' else (b, a)
    op = Alu.bitwise_and if kind == 'ero' else Alu.bitwise_or
    if not (up or dn):
        return src
    v = nc.vector
    U = src[:, 0:R, :]
    C = src[:, 1:R + 1, :]
    D = src[:, 2:R + 2, :]
    payload = dst[:, 1:R + 1, :] if dst.shape[1] == R + 2 else dst[:, :, :]
    if up and dn:
        v.tensor_tensor(payload, C, U, op)
        v.tensor_tensor(payload, payload, D, op)
    elif up:
        v.tensor_tensor(payload, C, U, op)
    else:
        v.tensor_tensor(payload, C, D, op)
    return dst


def _exchange_halo(nc, tl, top=True, bot=True):
    """Refresh halo rows of a [P, R+2, WW] tile via partition-shifted DMA.

    The two directions go on the two HWDGE queues so their ~1.7us init
    latencies overlap."""
    if top:
        # top halo j=0 of partition p = row 8p-1 = partition p-1's last payload j=8
        nc.sync.dma_start(tl[1:P, 0:1, :], tl[0:P - 1, R:R + 1, :])
    if bot:
        # bottom halo j=9 of partition p = row 8p+8 = partition p+1's first payload j=1
        nc.scalar.dma_start(tl[0:P - 1, R + 1:R + 2, :], tl[1:P, 1:2, :])



def _final_stage(nc, im):
    """result = (3x3 box count of out == 1) on packed words, into im.res."""
    v = nc.vector
    U = im.out[:, 0:R, :]
    C = im.out[:, 1:R + 1, :]
    D = im.out[:, 2:R + 2, :]
    v1 = im.x[:, 1:R + 1, :]
    v2 = im.ero[:, 1:R + 1, :]
    s1 = im.hm[:, 1:R + 1, :]
    q1 = im.vis[:, :, :]
    v.tensor_tensor(s1, U, C, Alu.bitwise_xor)
    v.tensor_tensor(q1, s1, D, Alu.bitwise_and)
    v.tensor_tensor(v1, s1, D, Alu.bitwise_xor)
    v.tensor_tensor(s1, U, C, Alu.bitwise_and)
    v.tensor_tensor(v2, s1, q1, Alu.bitwise_or)

    jlo, jhi = 1, R + 1
    S = (slice(None), slice(jlo, jhi), slice(None))
    v1L = im.hm[S]
    v.tensor_scalar(im.c[:, jlo:jhi, 1:WW], im.x[:, jlo:jhi, 0:WW - 1], 0, None, Alu.is_lt)
    _stt_i(v, v1L, v1, 1, im.c[S], Alu.logical_shift_left, Alu.bitwise_or)
    v1R = im.t[S]
    v.tensor_scalar(v1R, v1, 1, M31, Alu.logical_shift_right, Alu.bitwise_and)
    _stt_i(v, im.t[:, jlo:jhi, 0:WW - 1], im.x[:, jlo:jhi, 1:WW], 31,
           im.t[:, jlo:jhi, 0:WW - 1], Alu.logical_shift_left, Alu.bitwise_or)

    v2L = im.w[S]
    v.tensor_scalar(im.c[:, jlo:jhi, 1:WW], im.ero[:, jlo:jhi, 0:WW - 1], 0, None, Alu.is_lt)
    _stt_i(v, v2L, v2, 1, im.c[S], Alu.logical_shift_left, Alu.bitwise_or)
    v2R = im.u[S]
    v.tensor_scalar(v2R, v2, 1, M31, Alu.logical_shift_right, Alu.bitwise_and)
    _stt_i(v, im.u[:, jlo:jhi, 0:WW - 1], im.ero[:, jlo:jhi, 1:WW], 31,
           im.u[:, jlo:jhi, 0:WW - 1], Alu.logical_shift_left, Alu.bitwise_or)

    # any ">= 2" evidence: a v2 flag, or a pair among the v1 flags. With the
    # XOR3 parity, excluding just (v1L & v1) suffices: pairs kill the parity,
    # the triple is killed by the (v1L & v1) term.
    A = im.res[:, :, :]
    v.tensor_tensor(A, v2, v2L, Alu.bitwise_or)
    v.tensor_tensor(A, A, v2R, Alu.bitwise_or)
    P1 = im.vis[:, :, :]
    v.tensor_tensor(P1, v1L, v1, Alu.bitwise_and)
    v.tensor_tensor(A, A, P1, Alu.bitwise_or)
    X = im.hm[:, 1:R + 1, :]
    v.tensor_tensor(X, v1L, v1, Alu.bitwise_xor)
    v.tensor_tensor(X, X, v1R, Alu.bitwise_xor)
    _stt_i(v, im.res[:, :, :], A, -1, X, Alu.bitwise_xor, Alu.bitwise_and)


def build_program():
    nc = bacc.Bacc("TRN2", target_bir_lowering=False, debug=False, num_devices=N_CORES)
    x_in = nc.dram_tensor("x", [IMGS_PER_CORE, H, W], F32, kind="ExternalInput")
    y_out = nc.dram_tensor("y", [IMGS_PER_CORE, H, W], F32, kind="ExternalOutput")
    with tile.TileContext(nc) as tc:
        _emit(tc, x_in, y_out)
    nc.compile()
    return nc


def _emit(tc, x_in, y_out):
    nc = tc.nc
    if True:
        with tc.tile_pool(name="small", bufs=1) as pool, \
             tc.tile_pool(name="big", bufs=2) as big_pool, \
             tc.tile_pool(name="scratch", bufs=1) as scratch_pool:
            ims = [_Img(pool, big_pool, i) for i in range(IMGS_PER_CORE)]
            v = nc.vector

            # ---- one-time edge zeroing (image-boundary halo invariants) ----
            for im in ims:
                v.memset(im.x[0:32, 0, :], 0)
                v.memset(im.x[96:P, R + 1, :], 0)
                v.memset(im.ero[0:32, 0, :], 0)
                v.memset(im.ero[96:P, R + 1, :], 0)
                v.memset(im.out[:], 0)              # output accumulator + halos
                v.memset(im.c[:, :, 0:1], 0)        # L-carry left edge

            # ---- load (column-chunked) + pack, pipelined per chunk ----
            CHUNKS = [(c, c + 256) for c in range(0, W, 256)]
            NCH = len(CHUNKS)
            pk1s, pk2s, pk3s, pk4s = {}, {}, {}, {}
            for im in ims:
                pk1s[im] = scratch_pool.tile([P, R, W // 2], F32, tag=f"pka{im.idx}", name=f"pka{im.idx}")
                pk2s[im] = scratch_pool.tile([P, R, W // 4], F32, tag=f"pkb{im.idx}", name=f"pkb{im.idx}")
                pk3s[im] = scratch_pool.tile([P, R, W // 8], F32, tag=f"pkc{im.idx}", name=f"pkc{im.idx}")
                pk4s[im] = scratch_pool.tile([P, R, W // 16], I32, tag=f"pkd{im.idx}", name=f"pkd{im.idx}")
            for im, xi in zip(ims, range(IMGS_PER_CORE)):
                xr = x_in[xi].rearrange("(p j) c -> p j c", j=R)
                for ci, (c0, c1) in enumerate(CHUNKS):
                    eng = nc.sync if ci % 2 == 0 else nc.scalar
                    eng.dma_start(im.stage[:, :, c0:c1], xr[:, :, c0:c1])
                    st, pk1, pk2, pk3, pk4 = im.stage, pk1s[im], pk2s[im], pk3s[im], pk4s[im]
                    v.scalar_tensor_tensor(pk1[:, :, c0 // 2:c1 // 2], st[:, :, c0 + 1:c1:2], 2.0,
                                           st[:, :, c0:c1:2], Alu.mult, Alu.add)
                    v.scalar_tensor_tensor(pk2[:, :, c0 // 4:c1 // 4], pk1[:, :, c0 // 2 + 1:c1 // 2:2], 4.0,
                                           pk1[:, :, c0 // 2:c1 // 2:2], Alu.mult, Alu.add)
                    v.scalar_tensor_tensor(pk3[:, :, c0 // 8:c1 // 8], pk2[:, :, c0 // 4 + 1:c1 // 4:2], 16.0,
                                           pk2[:, :, c0 // 4:c1 // 4:2], Alu.mult, Alu.add)
                    # L4 writes int32 directly (arithmetic ops may cast)
                    v.scalar_tensor_tensor(pk4[:, :, c0 // 16:c1 // 16], pk3[:, :, c0 // 8 + 1:c1 // 8:2], 256.0,
                                           pk3[:, :, c0 // 8:c1 // 8:2], Alu.mult, Alu.add)
                    _stt_i(v, im.x[:, 1:R + 1, c0 // 32:c1 // 32], pk4[:, :, c0 // 16 + 1:c1 // 16:2], 16,
                           pk4[:, :, c0 // 16:c1 // 16:2], Alu.logical_shift_left, Alu.bitwise_or)
            for im in ims:
                _exchange_halo(nc, im.x)

            # ---- 8 sequential erosion/dilation iterations ----
            ero_ts = {}
            for k in range(8):
                vpat, hpat = KV[k], KH[k]
                # halo rows needed from the ero result by the dilation V-pass
                d_top, d_bot = (vpat[2], vpat[0]) if (vpat[0] or vpat[2]) else (0, 0)
                # halo rows of x needed by the NEXT iteration's erosion V-pass
                if k < 7:
                    nv = KV[k + 1]
                    x_top, x_bot = nv[0], nv[2]
                else:
                    x_top = x_bot = 0
                for im in ims:   # erosion phase (interleaved across images)
                    jlo, jhi = 1 - (vpat[0] or 0), R + 1 + (vpat[2] or 0)
                    hsrc = _hstage(nc, im, im.hm, im.x, hpat, 'ero', jlo, jhi)
                    esrc = _vstage(nc, im.ero, hsrc, vpat, 'ero')
                    ero_t = esrc if esrc is hsrc else im.ero
                    if ero_t is im.ero and (d_top or d_bot):
                        _exchange_halo(nc, ero_t, top=bool(d_top), bot=bool(d_bot))
                    ero_ts[im] = ero_t
                for im in ims:   # dilation + update phase
                    ero_t = ero_ts[im]
                    v.tensor_tensor(im.out[:, 1:R + 1, :], im.out[:, 1:R + 1, :],
                                    ero_t[:, 1:R + 1, :], Alu.bitwise_or)
                    if k == 7:
                        # out is final now; exchange its halos early so the
                        # final stage doesn't wait on the DMA init latency
                        _exchange_halo(nc, im.out)
                    jlo, jhi = 1 - (d_top or 0), R + 1 + (d_bot or 0)
                    dsrc = _hstage(nc, im, im.hm, ero_t, hpat, 'dil', jlo, jhi)
                    vsrc = _vstage(nc, im.vis, dsrc, vpat, 'dil')
                    vis_ap = vsrc[:, 1:R + 1, :] if vsrc.shape[1] == R + 2 else vsrc[:, :, :]
                    _stt_i(v, im.x[:, 1:R + 1, :], vis_ap, -1, im.x[:, 1:R + 1, :],
                           Alu.bitwise_xor, Alu.bitwise_and)
                    if x_top or x_bot:
                        _exchange_halo(nc, im.x, top=bool(x_top), bot=bool(x_bot))

            # ---- final (exactly-one of 3x3 box) + unpack/cast/store, per image ----
            for i, im in enumerate(ims):
                _final_stage(nc, im)
                stageI = scratch_pool.tile([P, R, W], I32, tag="stageI")
                # last image: small final chunk so its cast+store tail is short
                bounds = [0, 16, 32] if i == 0 else [0, 13, 24, 32]
                for half in range(len(bounds) - 1):
                    w0, w1 = bounds[half], bounds[half + 1]
                    c0, c1 = w0 * 32, w1 * 32
                    for b in range(32):
                        if b == 0:
                            v.tensor_scalar(stageI[:, :, c0::32][:, :, 0:w1 - w0], im.res[:, :, w0:w1],
                                            1, None, Alu.bitwise_and)
                        else:
                            v.tensor_scalar(stageI[:, :, c0 + b::32][:, :, 0:w1 - w0], im.res[:, :, w0:w1],
                                            b, 1, Alu.logical_shift_right, Alu.bitwise_and)
                    nc.scalar.copy(im.stage[:, :, c0:c1], stageI[:, :, c0:c1])
                    yr = y_out[i].rearrange("(p j) c -> p j c", j=R)
                    seng = nc.sync if half % 2 == 0 else nc.scalar
                    seng.dma_start(yr[:, :, c0:c1], im.stage[:, :, c0:c1])


_PROGRAM = None


def _get_program():
    global _PROGRAM
    if _PROGRAM is None:
        _PROGRAM = build_program()
    return _PROGRAM


def kernel(x: np.ndarray, kernels: np.ndarray = None, **_):
    """x: [16,1,1024,1024] fp32 binary -> [16,1,1024,1024] fp32."""
    x = np.ascontiguousarray(np.asarray(x), dtype=np.float32)
    N = x.shape[0]
    xs = x.reshape(N, H, W)
    nc = _get_program()
    in_maps = [{"x": np.ascontiguousarray(xs[c * IMGS_PER_CORE:(c + 1) * IMGS_PER_CORE])}
               for c in range(N_CORES)]
    res = run_bass_kernel_spmd(nc, in_maps, core_ids=list(range(N_CORES)))
    out = np.concatenate([r["y"] for r in res.results], axis=0)
    return out.reshape(N, 1, H, W).astype(np.float32)


# revision 33
# speedup vs baseline: 1.0120x; 1.0022x over previous
"""Trainium2 Bass kernel for nn_Erosion2d (sequential morphological erosion).

Bit-exact reformulation of the reference:
  x stays binary {0,1} throughout; every structuring element is rank-1
  (separable), so each erosion = separable AND (H-pass then V-pass) and each
  dilation = separable OR. The whole pipeline runs on 32x bit-packed words
  (int32; bit b of word (r,cw) = pixel (r, 32*cw+b)) using DVE bitwise ops.
  Final stage: result = (3x3 box count of output == 1), computed with
  carry-save bit tricks on packed words.

Layout per image: [128 partitions, j, 32 words]; partition p holds rows
8p..8p+7 at j=1..8 with halo rows j=0 (row 8p-1) and j=9 (row 8p+8).
V-taps are word-aligned free-dim offsets; halos are refreshed with small
SBUF->SBUF partition-shifted DMAs. H-taps are in-word funnel shifts.

Sharding: pure data parallel - 16 images, 2 per NeuronCore on 8 cores.
"""
import numpy as np

import concourse.bass as bass
import concourse.bacc as bacc
import concourse.mybir as mybir
import concourse.tile as tile
from concourse.bass_utils import run_bass_kernel_spmd

Alu = mybir.AluOpType
F32 = mybir.dt.float32
I32 = mybir.dt.int32

N_CORES = 8
IMGS_PER_CORE = 2
H = 1024
W = 1024
P = 128
R = H // P            # 8 rows per partition
WW = W // 32          # 32 words per row
M31 = 0x7FFFFFFF

# separable patterns (tap at -1, 0, +1) for the 8 structuring elements
KV = [(1, 1, 1), (1, 1, 1), (1, 1, 0), (0, 1, 1), (1, 1, 1), (0, 1, 0), (1, 1, 0), (0, 1, 0)]
KH = [(1, 1, 1), (0, 1, 1), (1, 1, 1), (0, 1, 1), (0, 1, 0), (1, 1, 1), (0, 1, 0), (1, 1, 0)]


def _stt_i(eng, out, in0, scalar, in1, op0, op1):
    """scalar_tensor_tensor with an int32-typed immediate."""
    return eng.add_instruction(
        mybir.InstTensorScalarPtr(
            name=eng.bass.get_next_instruction_name(),
            is_scalar_tensor_tensor=True,
            op0=op0, op1=op1,
            ins=[eng.lower_ap(in0),
                 eng.lower_ap_or_imm(scalar, imm_dtype=I32),
                 eng.lower_ap(in1)],
            outs=[eng.lower_ap(out)],
        ))


class _Img:
    """Per-image persistent tiles."""

    def __init__(self, pool, big_pool, idx):
        self.idx = idx
        dt = I32
        self.x = pool.tile([P, R + 2, WW], dt, tag=f"x{idx}")
        self.ero = pool.tile([P, R + 2, WW], dt, tag=f"ero{idx}")
        self.hm = pool.tile([P, R + 2, WW], dt, tag=f"hm{idx}")
        self.out = pool.tile([P, R + 2, WW], dt, tag=f"out{idx}")
        self.c = pool.tile([P, R + 2, WW], dt, tag=f"c{idx}")     # L-carries ([:,:,0]=0)
        self.t = pool.tile([P, R + 2, WW], dt, tag=f"t{idx}")     # scratch
        self.u = pool.tile([P, R + 2, WW], dt, tag=f"u{idx}")     # R-carry ([:,:,31]=0)
        self.w = pool.tile([P, R + 2, WW], dt, tag=f"w{idx}")     # scratch
        self.vis = pool.tile([P, R, WW], dt, tag=f"vis{idx}")
        self.res = pool.tile([P, R, WW], dt, tag=f"res{idx}")
        # big staging buffers are shared across images via tag rotation:
        # stage gets 2 slots (bufs=2 pool), the rest share single slots with
        # WAR deps inserted by Tile.
        self.stage = big_pool.tile([P, R, W], F32, tag="stage")


def _hstage(nc, im, dst, src, pat, kind, jlo=0, jhi=R + 2):
    """H-pass over j-slots [jlo, jhi): dst = src <&/|> shifted taps.

    'ero': dst = src & [L(src) if pat[0]] & [R(src) if pat[2]]
    'dil': dst = src | [R(src) if pat[0]] | [L(src) if pat[2]]
    L(x)[c] = x[c-1] = (w << 1) | (prev word bit31);
    R(x)[c] = x[c+1] = ((w >> 1) & M31) | (next word bit0 << 31).
    src is only read before the single dst write, so dst may alias src.
    Returns the result tile (src itself when the pattern is 010).
    """
    a, _, b = pat
    want_L, want_R = (a, b) if kind == 'ero' else (b, a)
    fold = Alu.bitwise_and if kind == 'ero' else Alu.bitwise_or
    if not (want_L or want_R):
        return src
    v = nc.vector
    S = (slice(None), slice(jlo, jhi), slice(None))
    chains = []
    if want_L:
        v.tensor_scalar(im.c[:, jlo:jhi, 1:WW], src[:, jlo:jhi, 0:WW - 1], 0, None, Alu.is_lt)
        _stt_i(v, im.t[S], src[S], 1, im.c[S], Alu.logical_shift_left, Alu.bitwise_or)
        chains.append(im.t)
    if want_R:
        hold = im.w if want_L else im.t
        v.tensor_scalar(hold[S], src[S], 1, M31, Alu.logical_shift_right, Alu.bitwise_and)
        # (next word << 31) isolates its bit0 at position 31 -- no mask needed;
        # last word untouched: its bit31 stays 0 (right image edge).
        _stt_i(v, hold[:, jlo:jhi, 0:WW - 1], src[:, jlo:jhi, 1:WW], 31,
               hold[:, jlo:jhi, 0:WW - 1], Alu.logical_shift_left, Alu.bitwise_or)
        chains.append(hold)
    if len(chains) == 2:
        v.tensor_tensor(im.t[S], im.t[S], im.w[S], fold)
    v.tensor_tensor(dst[S], chains[0][S], src[S], fold)
    return dst


def _vstage(nc, dst, src, pat, kind):
    """V-pass into payload rows (j=1..8 of a halo'd dst, or all of [P,R,WW]).

    'ero': AND over taps (pat[0] -> row-1, pat[2] -> row+1)
    'dil': OR over taps (pat[0] -> row+1, pat[2] -> row-1)
    src must have valid halo rows. Returns src itself when pattern is 010.
    """
    a, _, b = pat
    up, dn = (a, b) if kind == 'ero' else (b, a)
    op = Alu.bitwise_and if kind == 'ero' else Alu.bitwise_or
    if not (up or dn):
        return src
    v = nc.vector
    U = src[:, 0:R, :]
    C = src[:, 1:R + 1, :]
    D = src[:, 2:R + 2, :]
    payload = dst[:, 1:R + 1, :] if dst.shape[1] == R + 2 else dst[:, :, :]
    if up and dn:
        v.tensor_tensor(payload, C, U, op)
        v.tensor_tensor(payload, payload, D, op)
    elif up:
        v.tensor_tensor(payload, C, U, op)
    else:
        v.tensor_tensor(payload, C, D, op)
    return dst


def _exchange_halo(nc, tl, top=True, bot=True):
    """Refresh halo rows of a [P, R+2, WW] tile via partition-shifted DMA.

    The two directions go on the two HWDGE queues so their ~1.7us init
    latencies overlap."""
    if top:
        # top halo j=0 of partition p = row 8p-1 = partition p-1's last payload j=8
        nc.sync.dma_start(tl[1:P, 0:1, :], tl[0:P - 1, R:R + 1, :])
    if bot:
        # bottom halo j=9 of partition p = row 8p+8 = partition p+1's first payload j=1
        nc.scalar.dma_start(tl[0:P - 1, R + 1:R + 2, :], tl[1:P, 1:2, :])



def _final_stage(nc, im):
    """result = (3x3 box count of out == 1) on packed words, into im.res."""
    v = nc.vector
    U = im.out[:, 0:R, :]
    C = im.out[:, 1:R + 1, :]
    D = im.out[:, 2:R + 2, :]
    v1 = im.x[:, 1:R + 1, :]
    v2 = im.ero[:, 1:R + 1, :]
    s1 = im.hm[:, 1:R + 1, :]
    q1 = im.vis[:, :, :]
    v.tensor_tensor(s1, U, C, Alu.bitwise_xor)
    v.tensor_tensor(q1, s1, D, Alu.bitwise_and)
    v.tensor_tensor(v1, s1, D, Alu.bitwise_xor)
    v.tensor_tensor(s1, U, C, Alu.bitwise_and)
    v.tensor_tensor(v2, s1, q1, Alu.bitwise_or)

    jlo, jhi = 1, R + 1
    S = (slice(None), slice(jlo, jhi), slice(None))
    v1L = im.hm[S]
    v.tensor_scalar(im.c[:, jlo:jhi, 1:WW], im.x[:, jlo:jhi, 0:WW - 1], 0, None, Alu.is_lt)
    _stt_i(v, v1L, v1, 1, im.c[S], Alu.logical_shift_left, Alu.bitwise_or)
    v1R = im.t[S]
    v.tensor_scalar(v1R, v1, 1, M31, Alu.logical_shift_right, Alu.bitwise_and)
    _stt_i(v, im.t[:, jlo:jhi, 0:WW - 1], im.x[:, jlo:jhi, 1:WW], 31,
           im.t[:, jlo:jhi, 0:WW - 1], Alu.logical_shift_left, Alu.bitwise_or)

    v2L = im.w[S]
    v.tensor_scalar(im.c[:, jlo:jhi, 1:WW], im.ero[:, jlo:jhi, 0:WW - 1], 0, None, Alu.is_lt)
    _stt_i(v, v2L, v2, 1, im.c[S], Alu.logical_shift_left, Alu.bitwise_or)
    v2R = im.u[S]
    v.tensor_scalar(v2R, v2, 1, M31, Alu.logical_shift_right, Alu.bitwise_and)
    _stt_i(v, im.u[:, jlo:jhi, 0:WW - 1], im.ero[:, jlo:jhi, 1:WW], 31,
           im.u[:, jlo:jhi, 0:WW - 1], Alu.logical_shift_left, Alu.bitwise_or)

    # any ">= 2" evidence: a v2 flag, or a pair among the v1 flags. With the
    # XOR3 parity, excluding just (v1L & v1) suffices: pairs kill the parity,
    # the triple is killed by the (v1L & v1) term.
    A = im.res[:, :, :]
    v.tensor_tensor(A, v2, v2L, Alu.bitwise_or)
    v.tensor_tensor(A, A, v2R, Alu.bitwise_or)
    P1 = im.vis[:, :, :]
    v.tensor_tensor(P1, v1L, v1, Alu.bitwise_and)
    v.tensor_tensor(A, A, P1, Alu.bitwise_or)
    X = im.hm[:, 1:R + 1, :]
    v.tensor_tensor(X, v1L, v1, Alu.bitwise_xor)
    v.tensor_tensor(X, X, v1R, Alu.bitwise_xor)
    _stt_i(v, im.res[:, :, :], A, -1, X, Alu.bitwise_xor, Alu.bitwise_and)


def build_program():
    nc = bacc.Bacc("TRN2", target_bir_lowering=False, debug=False, num_devices=N_CORES)
    x_in = nc.dram_tensor("x", [IMGS_PER_CORE, H, W], F32, kind="ExternalInput")
    y_out = nc.dram_tensor("y", [IMGS_PER_CORE, H, W], F32, kind="ExternalOutput")
    with tile.TileContext(nc) as tc:
        _emit(tc, x_in, y_out)
    nc.compile()
    return nc


def _emit(tc, x_in, y_out):
    nc = tc.nc
    if True:
        with tc.tile_pool(name="small", bufs=1) as pool, \
             tc.tile_pool(name="big", bufs=2) as big_pool, \
             tc.tile_pool(name="scratch", bufs=1) as scratch_pool:
            ims = [_Img(pool, big_pool, i) for i in range(IMGS_PER_CORE)]
            v = nc.vector

            # ---- one-time edge zeroing (image-boundary halo invariants) ----
            for im in ims:
                v.memset(im.x[0:32, 0, :], 0)
                v.memset(im.x[96:P, R + 1, :], 0)
                v.memset(im.ero[0:32, 0, :], 0)
                v.memset(im.ero[96:P, R + 1, :], 0)
                v.memset(im.out[0:32, 0, :], 0)     # out image-edge halo rows
                v.memset(im.out[96:P, R + 1, :], 0)
                v.memset(im.c[:, :, 0:1], 0)        # L-carry left edge

            # ---- load (column-chunked) + pack, pipelined per chunk ----
            CHUNKS = [(c, c + 256) for c in range(0, W, 256)]
            NCH = len(CHUNKS)
            pk1s, pk2s, pk3s, pk4s = {}, {}, {}, {}
            for im in ims:
                pk1s[im] = scratch_pool.tile([P, R, W // 2], F32, tag=f"pka{im.idx}", name=f"pka{im.idx}")
                pk2s[im] = scratch_pool.tile([P, R, W // 4], F32, tag=f"pkb{im.idx}", name=f"pkb{im.idx}")
                pk3s[im] = scratch_pool.tile([P, R, W // 8], F32, tag=f"pkc{im.idx}", name=f"pkc{im.idx}")
                pk4s[im] = scratch_pool.tile([P, R, W // 16], I32, tag=f"pkd{im.idx}", name=f"pkd{im.idx}")
            for im, xi in zip(ims, range(IMGS_PER_CORE)):
                xr = x_in[xi].rearrange("(p j) c -> p j c", j=R)
                for ci, (c0, c1) in enumerate(CHUNKS):
                    eng = nc.sync if ci % 2 == 0 else nc.scalar
                    eng.dma_start(im.stage[:, :, c0:c1], xr[:, :, c0:c1])
                    st, pk1, pk2, pk3, pk4 = im.stage, pk1s[im], pk2s[im], pk3s[im], pk4s[im]
                    v.scalar_tensor_tensor(pk1[:, :, c0 // 2:c1 // 2], st[:, :, c0 + 1:c1:2], 2.0,
                                           st[:, :, c0:c1:2], Alu.mult, Alu.add)
                    v.scalar_tensor_tensor(pk2[:, :, c0 // 4:c1 // 4], pk1[:, :, c0 // 2 + 1:c1 // 2:2], 4.0,
                                           pk1[:, :, c0 // 2:c1 // 2:2], Alu.mult, Alu.add)
                    v.scalar_tensor_tensor(pk3[:, :, c0 // 8:c1 // 8], pk2[:, :, c0 // 4 + 1:c1 // 4:2], 16.0,
                                           pk2[:, :, c0 // 4:c1 // 4:2], Alu.mult, Alu.add)
                    # L4 writes int32 directly (arithmetic ops may cast)
                    v.scalar_tensor_tensor(pk4[:, :, c0 // 16:c1 // 16], pk3[:, :, c0 // 8 + 1:c1 // 8:2], 256.0,
                                           pk3[:, :, c0 // 8:c1 // 8:2], Alu.mult, Alu.add)
                    _stt_i(v, im.x[:, 1:R + 1, c0 // 32:c1 // 32], pk4[:, :, c0 // 16 + 1:c1 // 16:2], 16,
                           pk4[:, :, c0 // 16:c1 // 16:2], Alu.logical_shift_left, Alu.bitwise_or)
            for im in ims:
                _exchange_halo(nc, im.x)

            # ---- 8 sequential erosion/dilation iterations ----
            ero_ts = {}
            for k in range(8):
                vpat, hpat = KV[k], KH[k]
                # halo rows needed from the ero result by the dilation V-pass
                d_top, d_bot = (vpat[2], vpat[0]) if (vpat[0] or vpat[2]) else (0, 0)
                # halo rows of x needed by the NEXT iteration's erosion V-pass
                if k < 7:
                    nv = KV[k + 1]
                    x_top, x_bot = nv[0], nv[2]
                else:
                    x_top = x_bot = 0
                for im in ims:   # erosion phase (interleaved across images)
                    jlo, jhi = 1 - (vpat[0] or 0), R + 1 + (vpat[2] or 0)
                    hsrc = _hstage(nc, im, im.hm, im.x, hpat, 'ero', jlo, jhi)
                    esrc = _vstage(nc, im.ero, hsrc, vpat, 'ero')
                    ero_t = esrc if esrc is hsrc else im.ero
                    if ero_t is im.ero and (d_top or d_bot):
                        _exchange_halo(nc, ero_t, top=bool(d_top), bot=bool(d_bot))
                    ero_ts[im] = ero_t
                for im in ims:   # dilation + update phase
                    ero_t = ero_ts[im]
                    if k == 0:
                        # out is all-zero: plain copy runs at 2x vs 1x TT
                        v.tensor_copy(im.out[:, 1:R + 1, :], ero_t[:, 1:R + 1, :])
                    else:
                        v.tensor_tensor(im.out[:, 1:R + 1, :], im.out[:, 1:R + 1, :],
                                        ero_t[:, 1:R + 1, :], Alu.bitwise_or)
                    if k == 7:
                        # out is final now; exchange its halos early so the
                        # final stage doesn't wait on the DMA init latency
                        _exchange_halo(nc, im.out)
                    jlo, jhi = 1 - (d_top or 0), R + 1 + (d_bot or 0)
                    dsrc = _hstage(nc, im, im.hm, ero_t, hpat, 'dil', jlo, jhi)
                    vsrc = _vstage(nc, im.vis, dsrc, vpat, 'dil')
                    vis_ap = vsrc[:, 1:R + 1, :] if vsrc.shape[1] == R + 2 else vsrc[:, :, :]
                    _stt_i(v, im.x[:, 1:R + 1, :], vis_ap, -1, im.x[:, 1:R + 1, :],
                           Alu.bitwise_xor, Alu.bitwise_and)
                    if x_top or x_bot:
                        _exchange_halo(nc, im.x, top=bool(x_top), bot=bool(x_bot))

            # ---- final (exactly-one of 3x3 box) + unpack/cast/store, per image ----
            for i, im in enumerate(ims):
                _final_stage(nc, im)
                stageI = scratch_pool.tile([P, R, W], I32, tag="stageI")
                # last image: small final chunk so its cast+store tail is short
                bounds = [0, 16, 32] if i == 0 else [0, 13, 24, 32]
                for half in range(len(bounds) - 1):
                    w0, w1 = bounds[half], bounds[half + 1]
                    c0, c1 = w0 * 32, w1 * 32
                    for b in range(32):
                        if b == 0:
                            v.tensor_scalar(stageI[:, :, c0::32][:, :, 0:w1 - w0], im.res[:, :, w0:w1],
                                            1, None, Alu.bitwise_and)
                        else:
                            v.tensor_scalar(stageI[:, :, c0 + b::32][:, :, 0:w1 - w0], im.res[:, :, w0:w1],
                                            b, 1, Alu.logical_shift_right, Alu.bitwise_and)
                    nc.scalar.copy(im.stage[:, :, c0:c1], stageI[:, :, c0:c1])
                    yr = y_out[i].rearrange("(p j) c -> p j c", j=R)
                    seng = nc.sync if half % 2 == 0 else nc.scalar
                    seng.dma_start(yr[:, :, c0:c1], im.stage[:, :, c0:c1])


_PROGRAM = None


def _get_program():
    global _PROGRAM
    if _PROGRAM is None:
        _PROGRAM = build_program()
    return _PROGRAM


def kernel(x: np.ndarray, kernels: np.ndarray = None, **_):
    """x: [16,1,1024,1024] fp32 binary -> [16,1,1024,1024] fp32."""
    x = np.ascontiguousarray(np.asarray(x), dtype=np.float32)
    N = x.shape[0]
    xs = x.reshape(N, H, W)
    nc = _get_program()
    in_maps = [{"x": np.ascontiguousarray(xs[c * IMGS_PER_CORE:(c + 1) * IMGS_PER_CORE])}
               for c in range(N_CORES)]
    res = run_bass_kernel_spmd(nc, in_maps, core_ids=list(range(N_CORES)))
    out = np.concatenate([r["y"] for r in res.results], axis=0)
    return out.reshape(N, 1, H, W).astype(np.float32)
